# revision 3
# baseline (speedup 1.0000x reference)
"""Keras-GRU (reset_after=True) forward as a Bass/Tile kernel on 8 trn2 cores.

Problem: B=16, T=4096, D=512, H=512 (fp32 inputs).
Sharding: data-parallel over batch -> 2 sequences per core, weights replicated.

Per-core layout is fully "hidden-major" (transposed): every on-chip tensor
keeps a hidden-dim chunk of 128 on partitions and (t, b) pairs on the free
dim.  That makes all per-step elementwise/activation work [128, small] (full
lane utilization) and makes the recurrent matmul U-stationary:

  rec^T[mc] = sum_kc U[kc, mc]^T @ h^T[kc]     (48 self-loading MMs / step)

The input projection x@W runs as a blocked GEMM (256 steps at a time) into
PSUM and is evacuated (with input bias folded in via the ACT per-partition
bias) into an SBUF buffer read by the recurrence loop.

Host side pre-transposes x into [P, KC, 2T] (col = t*2+b) so the device never
transposes anything; the output is written hidden-major to DRAM and
un-transposed on the host.
"""

import os
from contextlib import ExitStack

import numpy as np

B, T, D, H = 16, 4096, 512, 512
NCORES = 8
P = 128
KC = D // P        # 4  input-dim chunks
HC = H // P        # 4  hidden-dim chunks
MC = 3 * H // P    # 12 gate-dim chunks (z: 0-3, r: 4-7, h: 8-11)

SBS = 256          # recurrence steps per super-block (xk staging granularity)
UNROLL = 2         # steps per For_i body (must divide SBS; body fits IRAM)

# "bf16" (fast) or "f32" (exact matmuls, ~2x slower recurrence)
MM_DTYPE = os.environ.get("GRU_MM_DTYPE", "bf16")
# 0: plain 128x128 stationary tiles; 1: 4x column-tiled 32-wide tiles
COL_TILE = int(os.environ.get("GRU_COL_TILE", "0"))

_PROGRAM_CACHE = {}


def build_program(t_steps=T, sbs=SBS, unroll=UNROLL, mm_dtype=MM_DTYPE,
                  col_tile=COL_TILE, debug=False):
    """Build + compile the per-core Bass program (same program on all cores)."""
    import concourse.mybir as mybir
    import concourse.tile as tile
    from concourse import bacc
    from concourse.bass import ds

    f32 = mybir.dt.float32
    bf16 = mybir.dt.bfloat16
    mmdt = bf16 if mm_dtype == "bf16" else f32
    AFT = mybir.ActivationFunctionType

    assert t_steps % sbs == 0 and sbs % unroll == 0
    nsb = t_steps // sbs
    cols = 2 * sbs                      # (t, b) columns per super-block

    nc = bacc.Bacc("TRN2", target_bir_lowering=False, debug=debug,
                   num_devices=NCORES)

    xT = nc.dram_tensor("xT", [P, KC, 2 * t_steps], mmdt,
                        kind="ExternalInput").ap()
    Uw = nc.dram_tensor("Uw", [HC, P, 3 * H], mmdt, kind="ExternalInput").ap()
    Ww = nc.dram_tensor("Ww", [KC, P, 3 * H], mmdt, kind="ExternalInput").ap()
    bev = nc.dram_tensor("bev", [P, MC], f32, kind="ExternalInput").ap()
    brec = nc.dram_tensor("brec", [P, 2 * HC], f32, kind="ExternalInput").ap()
    outT = nc.dram_tensor("outT", [P, t_steps * 2 * HC], f32,
                          kind="ExternalOutput").ap()

    with tile.TileContext(nc) as tc, ExitStack() as ctx:
        const = ctx.enter_context(tc.tile_pool(name="const", bufs=1))
        persist = ctx.enter_context(tc.tile_pool(name="persist", bufs=1))
        xtp = ctx.enter_context(tc.tile_pool(name="xtp", bufs=2))
        xkp = ctx.enter_context(tc.tile_pool(name="xkp", bufs=2))
        stp = ctx.enter_context(tc.tile_pool(name="stp", bufs=2))
        gps = ctx.enter_context(tc.tile_pool(name="gps", bufs=2, space="PSUM"))
        rps = ctx.enter_context(tc.tile_pool(name="rps", bufs=2, space="PSUM"))
        scr = ctx.enter_context(tc.tile_pool(name="scr", bufs=3))

        # ---- constants ----
        u_sb = const.tile([P, HC, 3 * H], mmdt)
        w_sb = const.tile([P, KC, 3 * H], mmdt)
        for k in range(HC):
            nc.sync.dma_start(out=u_sb[:, k, :], in_=Uw[k])
        for k in range(KC):
            nc.sync.dma_start(out=w_sb[:, k, :], in_=Ww[k])
        bev_sb = const.tile([P, MC], f32)
        nc.sync.dma_start(out=bev_sb, in_=bev)
        brec_sb = const.tile([P, 2 * HC], f32)
        nc.sync.dma_start(out=brec_sb, in_=brec)

        # ---- persistent GRU state (ping-pong, hidden-major [128, (kc, b)]) ----
        h_pp = [persist.tile([P, 2 * HC], f32, tag=f"h{i}", name=f"h{i}")
                for i in range(2)]
        hb_pp = [persist.tile([P, 2 * HC], mmdt, tag=f"hb{i}", name=f"hb{i}")
                 for i in range(2)]
        nc.vector.memset(h_pp[0], 0.0)
        nc.vector.memset(hb_pp[0], 0.0)

        for s in range(nsb):
            # ---- load x^T slice for this super-block ----
            xt_t = xtp.tile([P, KC, cols], mmdt)
            nc.sync.dma_start(out=xt_t, in_=xT[:, :, s * cols:(s + 1) * cols])

            # ---- blocked input GEMM: xk = x@W + bias  (hidden-major) ----
            # xk_t[p, dt, mc*2+b] = (x@W + bev)[b, s*sbs+dt, mc*128+p]
            xk_t = xkp.tile([P, sbs, 2 * MC], f32)
            for mc in range(MC):
                ps = gps.tile([P, cols], f32)
                for k in range(KC):
                    nc.tensor.matmul(
                        out=ps,
                        lhsT=w_sb[:, k, mc * P:(mc + 1) * P],
                        rhs=xt_t[:, k, :],
                        start=(k == 0), stop=(k == KC - 1),
                    )
                nc.scalar.activation(
                    out=xk_t[:, :, 2 * mc:2 * mc + 2],
                    in_=ps.rearrange("p (t b) -> p t b", b=2),
                    func=AFT.Identity,
                    bias=bev_sb[:, mc:mc + 1],
                )

            stag = stp.tile([P, sbs * 2 * HC], f32)

            # ---- sequential recurrence over this super-block ----
            with tc.For_i(0, sbs, unroll, staggered_reset=True,
                          hint_engines=(mybir.EngineType.PE,
                                        mybir.EngineType.DVE,
                                        mybir.EngineType.Activation)) as iv:
                for u in range(unroll):
                    h_in, h_out = h_pp[u % 2], h_pp[(u + 1) % 2]
                    hb_in, hb_out = hb_pp[u % 2], hb_pp[(u + 1) % 2]
                    dt = iv + u

                    # rec^T = U^T h  (+ gate-h region WITHOUT xk)
                    rp = rps.tile([P, 2 * MC], f32)
                    for mc in range(MC):
                        if col_tile:
                            for j in range(4):
                                for k in range(HC):
                                    nc.tensor.matmul(
                                        out=rp[32 * j:32 * (j + 1),
                                               2 * mc:2 * mc + 2],
                                        lhsT=u_sb[:, k,
                                                  mc * P + 32 * j:
                                                  mc * P + 32 * (j + 1)],
                                        rhs=hb_in[:, 2 * k:2 * k + 2],
                                        start=(k == 0), stop=(k == HC - 1),
                                        tile_position=(0, 32 * j),
                                    )
                        else:
                            for k in range(HC):
                                nc.tensor.matmul(
                                    out=rp[:, 2 * mc:2 * mc + 2],
                                    lhsT=u_sb[:, k, mc * P:(mc + 1) * P],
                                    rhs=hb_in[:, 2 * k:2 * k + 2],
                                    start=(k == 0), stop=(k == HC - 1),
                                )

                    xks = xk_t[:, ds(dt, 1), :]          # [P, 1, 24]
                    # z|r pre-activation: rec_zr + xk_zr
                    szr = scr.tile([P, 4 * HC], f32, tag="szr")
                    nc.vector.tensor_add(szr, rp[:, 0:4 * HC], xks[:, 0, 0:4 * HC])
                    # gate-h recurrent term + recurrent bias
                    vv = scr.tile([P, 2 * HC], f32, tag="vv")
                    nc.vector.tensor_add(vv, rp[:, 4 * HC:6 * HC], brec_sb)
                    # z,r gates
                    zr = scr.tile([P, 4 * HC], f32, tag="zr")
                    nc.scalar.activation(out=zr, in_=szr, func=AFT.Sigmoid)
                    # hh = tanh(xk_h + r * vv)
                    uu = scr.tile([P, 2 * HC], f32, tag="uu")
                    nc.vector.tensor_mul(uu, zr[:, 2 * HC:4 * HC], vv)
                    ww = scr.tile([P, 2 * HC], f32, tag="ww")
                    nc.vector.tensor_add(ww, uu, xks[:, 0, 4 * HC:6 * HC])
                    hh = scr.tile([P, 2 * HC], f32, tag="hh")
                    nc.scalar.activation(out=hh, in_=ww, func=AFT.Tanh)
                    # h' = z*h + (1-z)*hh = z*(h-hh) + hh
                    dd = scr.tile([P, 2 * HC], f32, tag="dd")
                    nc.vector.tensor_sub(dd, h_in, hh)
                    ee = scr.tile([P, 2 * HC], f32, tag="ee")
                    nc.vector.tensor_mul(ee, zr[:, 0:2 * HC], dd)
                    nc.vector.tensor_add(h_out, ee, hh)
                    # bf16 copy for next step's matmul rhs
                    nc.vector.tensor_copy(hb_out, h_out)
                    # stage output (ScalarE, off the DVE critical path)
                    nc.scalar.copy(stag[:, ds(dt * (2 * HC), 2 * HC)], h_out)

            nc.sync.dma_start(
                out=outT[:, s * sbs * 2 * HC:(s + 1) * sbs * 2 * HC],
                in_=stag,
            )

    nc.compile()
    return nc


def _get_program():
    key = (T, SBS, UNROLL, MM_DTYPE, COL_TILE)
    if key not in _PROGRAM_CACHE:
        _PROGRAM_CACHE[key] = build_program()
    return _PROGRAM_CACHE[key]


def make_in_maps(x, kernel_w, recurrent_kernel, bias, t_steps=T,
                 mm_dtype=MM_DTYPE, n_cores=NCORES):
    """Host-side shard + transpose + cast into the program's input layout."""
    import ml_dtypes

    mmdt = ml_dtypes.bfloat16 if mm_dtype == "bf16" else np.float32
    x = np.asarray(x, np.float32)
    kernel_w = np.asarray(kernel_w, np.float32)
    recurrent_kernel = np.asarray(recurrent_kernel, np.float32)
    bias = np.asarray(bias, np.float32)
    b0, b1 = bias[0], bias[1]

    Uw = np.ascontiguousarray(
        recurrent_kernel.astype(mmdt).reshape(HC, P, 3 * H))
    Ww = np.ascontiguousarray(kernel_w.astype(mmdt).reshape(KC, P, 3 * H))
    bev_full = b0 + np.concatenate([b1[:2 * H], np.zeros(H, np.float32)])
    bev = np.ascontiguousarray(bev_full.reshape(MC, P).T, dtype=np.float32)
    brec = np.ascontiguousarray(
        np.repeat(b1[2 * H:].reshape(HC, P).T[:, :, None], 2, axis=2)
        .reshape(P, 2 * HC), dtype=np.float32)

    bpc = x.shape[0] // n_cores
    assert bpc == 2, "layout hardcodes 2 sequences per core"
    in_maps = []
    for c in range(n_cores):
        xc = x[bpc * c:bpc * (c + 1)]                  # [2, T, D]
        xt = xc.astype(mmdt).transpose(2, 1, 0)        # [D, T, 2]
        xt = (xt.reshape(KC, P, t_steps, 2)
                .transpose(1, 0, 2, 3)
                .reshape(P, KC, 2 * t_steps))
        in_maps.append({"xT": np.ascontiguousarray(xt), "Uw": Uw, "Ww": Ww,
                        "bev": bev, "brec": brec})
    return in_maps


def unpack_out(outT_np, t_steps=T):
    """[P, T*2*HC] hidden-major staging layout -> [2, T, H]."""
    o = outT_np.reshape(P, t_steps, HC, 2).transpose(3, 1, 2, 0)
    return np.ascontiguousarray(o.reshape(2, t_steps, H), dtype=np.float32)


def kernel(x, kernel, recurrent_kernel, bias):
    from concourse.bass_utils import run_bass_kernel_spmd

    nc = _get_program()
    in_maps = make_in_maps(x, kernel, recurrent_kernel, bias)
    res = run_bass_kernel_spmd(nc, in_maps, core_ids=list(range(NCORES)))
    outputs = np.concatenate(
        [unpack_out(res.results[c]["outT"]) for c in range(NCORES)], axis=0)
    state = np.ascontiguousarray(outputs[:, -1, :])
    return outputs, state
